# revision 13
# baseline (speedup 1.0000x reference)
"""Trainium2 Bass kernel for nn_PredictionHead (MLP + segment softmax).

Strategy (8 NeuronCores, data-parallel over nodes):
  - Shard the 500k nodes at segment-id boundaries (segments [256c, 256(c+1))
    go to core c) so every segment's rows live on exactly one core.
  - Each core computes in the TRANSPOSED domain (features/classes on the
    partition axis, nodes on the free axis), so the MLP matmuls need no
    on-chip transposes: the host supplies H^T once, cast to bf16.
      h^T   = relu(W1^T @ H^T + b1)        (bf16 matmuls, f32 PSUM)
      ex^T  = exp(W2^T @ h^T + b2)         (ACT Exp, bf16 out)
  - The kernel outputs ex^T (bf16); the host recovers logits = ln(ex)
    exactly enough (|d logit| ~ 2^-9) and skips an on-chip bias-add pass.
  - Segment sums of ex along the node axis are computed with masked
    tensor_tensor_scan ops on the vector engine (segmented prefix sum,
    then a backward masked max-scan that spreads each segment's total over
    the segment), chained across 1024-column blocks; a one-block-lag carry
    fixes segments that straddle a block boundary.
  - rc = 1/spread on the scalar engine (LUT reciprocal, ~1e-6 rel);
    probs^T = ex^T * rc on gpsimd (bf16).
  - Host un-transposes the two outputs and concatenates the shards.

Engine budget per 1024-column block (measured costs):
  DVE    scan1 2.35us + scan2 3.02us            = 5.4us   <- critical path
  ACT    relu 2x1.12 + exp 1.12 + recip 1.12    = 4.5us
  GpSimd mask bcast 2.3 + mul 2.3 + copy 0.45   = 5.1us
  PE     12 matmul x 512 rows + 6 ldweights     ~ 3.5-4us
  DMA    ~1.1 MB                                ~ 2.9us
"""

import os
import sys

import numpy as np

if "/opt/trn_rl_repo" not in sys.path:
    sys.path.insert(0, "/opt/trn_rl_repo")

# Make sure the axon (neuron) PJRT platform stays reachable even if the
# embedding process pinned JAX_PLATFORMS=cpu for the jax reference.
_jp = os.environ.get("JAX_PLATFORMS")
if _jp and "axon" not in _jp and "jax" not in sys.modules:
    os.environ["JAX_PLATFORMS"] = _jp + ",axon"

N_NODES = 500_000
FEAT = 256
CLS = 128
NUM_SEGMENTS = 2048
NCORES = 8
SEG_PER_CORE = NUM_SEGMENTS // NCORES
B = 512       # matmul / PSUM bank sub-block width
BB = 1024     # columns processed per iteration
MPAD = 63488  # 62 iterations of 1024; max shard for the reference seed is 62846
NB = MPAD // BB
WIN = 352     # backward-scan lookahead; must exceed the max segment length
W = BB + WIN

_NC_CACHE = {}


def _recip_fast(nc, out, in_):
    """reciprocal_approx_fast with a bf16 output (bypasses the f32-out
    assert; the DVE write stage downcasts). ~51 ULP in f32, so the bf16
    rounding dominates -- fine at this problem's 2e-2 tolerance."""
    from concourse.dve_ops import (
        RECIP_APPROX_FAST_CONSTS,
        RECIPROCAL_APPROX_FAST,
    )

    c = RECIP_APPROX_FAST_CONSTS
    return nc.vector._custom_dve(
        RECIPROCAL_APPROX_FAST,
        out=out,
        in0=in_,
        s0=c["s0"],
        s1=c["s1"],
        imm2=c["imm2"],
    )


def _build_nc(mul_on_gpsimd=True):
    from contextlib import ExitStack

    import concourse.bacc as bacc
    import concourse.mybir as mybir
    import concourse.tile as tile

    f32 = mybir.dt.float32
    bf16 = mybir.dt.bfloat16
    AF = mybir.ActivationFunctionType
    OP = mybir.AluOpType

    nc = bacc.Bacc("TRN2", target_bir_lowering=False, debug=False)
    # ht is laid out [2, 128, MPAD]: the two 128-row k-chunks of H^T stacked,
    # so one DMA per iteration fetches both chunks of a column block.
    ht_d = nc.dram_tensor("ht", [2, 128, MPAD], bf16, kind="ExternalInput")
    w1_d = nc.dram_tensor("w1", [FEAT, FEAT], bf16, kind="ExternalInput")
    w2_d = nc.dram_tensor("w2", [FEAT, CLS], bf16, kind="ExternalInput")
    b1_d = nc.dram_tensor("b1", [FEAT, 1], f32, kind="ExternalInput")
    b2_d = nc.dram_tensor("b2", [CLS, 1], f32, kind="ExternalInput")
    # mask ships pre-broadcast to 128 partitions (bf16, tiny vs H) so no
    # on-chip partition broadcast is needed.
    cm_d = nc.dram_tensor("cm", [128, MPAD + W + 1], bf16, kind="ExternalInput")
    eo_d = nc.dram_tensor("eo", [CLS, MPAD], bf16, kind="ExternalOutput")
    pt_d = nc.dram_tensor("pt", [CLS, MPAD], bf16, kind="ExternalOutput")

    with ExitStack() as ctx:
        tc = ctx.enter_context(tile.TileContext(nc))
        consts = ctx.enter_context(tc.tile_pool(name="consts", bufs=1))
        htp = ctx.enter_context(tc.tile_pool(name="htp", bufs=4))
        hp = ctx.enter_context(tc.tile_pool(name="hp", bufs=6))
        psh = ctx.enter_context(tc.tile_pool(name="psh", bufs=1, space="PSUM"))
        psl = ctx.enter_context(tc.tile_pool(name="psl", bufs=2, space="PSUM"))
        exq = ctx.enter_context(tc.tile_pool(name="exq", bufs=4))
        csp = ctx.enter_context(tc.tile_pool(name="csp", bufs=3))
        mbc = ctx.enter_context(tc.tile_pool(name="mbc", bufs=4))
        prescan = ctx.enter_context(tc.tile_pool(name="prescan", bufs=2))
        xpp = ctx.enter_context(tc.tile_pool(name="xpp", bufs=3))
        rcp = ctx.enter_context(tc.tile_pool(name="rcp", bufs=3))
        prp = ctx.enter_context(tc.tile_pool(name="prp", bufs=3))

        w1k0 = consts.tile([128, FEAT], bf16)
        nc.sync.dma_start(w1k0[:], w1_d.ap()[0:128, :])
        w1k1 = consts.tile([128, FEAT], bf16)
        nc.sync.dma_start(w1k1[:], w1_d.ap()[128:256, :])
        w2k0 = consts.tile([128, CLS], bf16)
        nc.sync.dma_start(w2k0[:], w2_d.ap()[0:128, :])
        w2k1 = consts.tile([128, CLS], bf16)
        nc.sync.dma_start(w2k1[:], w2_d.ap()[128:256, :])
        b1a = consts.tile([128, 1], f32)
        nc.sync.dma_start(b1a[:], b1_d.ap()[0:128, :])
        b1b = consts.tile([128, 1], f32)
        nc.sync.dma_start(b1b[:], b1_d.ap()[128:256, :])
        b2t = consts.tile([128, 1], f32)
        nc.sync.dma_start(b2t[:], b2_d.ap()[:, :])

        def emit_tail(p, nxt):
            # Backward masked max-scan spreads each segment's total (csum at
            # its last column) over the segment. The carry INTO this block's
            # last column comes from a short pre-scan over the first WIN
            # columns of the NEXT block; it is injected as an extra data
            # element (cs column BB) rather than via `initial` -- a reversed
            # scan with an AP initial runs ~1.7x slower on hardware.
            if nxt is not None:
                pre = prescan.tile([128, WIN], f32)
                nc.vector.tensor_tensor_scan(
                    out=pre[:][:, ::-1],
                    data0=nxt["cmb"][:][:, 1 : WIN + 1][:, ::-1],
                    data1=nxt["cs"][:][:, 0:WIN][:, ::-1],
                    initial=0.0,
                    op0=OP.mult,
                    op1=OP.max,
                )
                nc.vector.tensor_copy(p["cs"][:][:, BB : BB + 1], pre[:][:, 0:1])
            else:
                nc.vector.memset(p["cs"][:][:, BB : BB + 1], 0.0)
            xpd = xpp.tile([128, BB + 1], f32)
            nc.vector.tensor_tensor_scan(
                out=xpd[:][:, ::-1],
                data0=p["cmb"][:][:, 1 : BB + 2][:, ::-1],
                data1=p["cs"][:][:, ::-1],
                initial=0.0,
                op0=OP.mult,
                op1=OP.max,
            )
            rc = rcp.tile([128, BB], bf16)
            _recip_fast(nc, rc[:], xpd[:][:, 0:BB])
            pr = prp.tile([128, BB], bf16)
            if mul_on_gpsimd:
                nc.gpsimd.tensor_mul(pr[:], p["ex"][:], rc[:])
            else:
                nc.vector.tensor_mul(pr[:], p["ex"][:], rc[:])
            nc.sync.dma_start(pt_d.ap()[:, p["mb"] : p["mb"] + BB], pr[:])

        # Software-pipelined schedule: in iteration i the tensor engine runs
        # MM1(i) then MM2(i-1) -- MM2's relu dependency is already satisfied,
        # so the PE never head-of-line blocks (keeps the HAM clock warm).
        st = {}

        def stage1(b):
            mb = b * BB
            htb = htp.tile([128, 2, BB], bf16)
            nc.sync.dma_start(
                htb[:], ht_d.ap()[:, :, mb : mb + BB].rearrange("k p m -> p k m")
            )
            cmb = mbc.tile([128, BB + 2], bf16)
            nc.sync.dma_start(cmb[:], cm_d.ap()[:, mb : mb + BB + 2])
            ht0 = htb[:][:, 0, :]
            ht1 = htb[:][:, 1, :]
            ph0 = psh.tile([128, BB], f32, tag="ph0")
            ph1 = psh.tile([128, BB], f32, tag="ph1")
            for c, ph in ((0, ph0), (1, ph1)):
                cs_ = slice(128 * c, 128 * (c + 1))
                for k, (wk, htk) in enumerate(((w1k0, ht0), (w1k1, ht1))):
                    for s in range(2):
                        sl = slice(s * B, (s + 1) * B)
                        nc.tensor.matmul(
                            ph[:][:, sl], wk[:][:, cs_], htk[:, sl],
                            start=k == 0, stop=k == 1,
                        )
            h0 = hp.tile([128, BB], bf16)
            nc.scalar.activation(h0[:], ph0[:], AF.Relu, bias=b1a[:])
            h1 = hp.tile([128, BB], bf16)
            nc.scalar.activation(h1[:], ph1[:], AF.Relu, bias=b1b[:])
            st[b] = dict(h0=h0, h1=h1, cmb=cmb, mb=mb)

        def stage2(b):
            p = st[b]
            pl = psl.tile([128, BB], f32)
            for k, (wk, hk) in enumerate(((w2k0, p["h0"]), (w2k1, p["h1"]))):
                for s in range(2):
                    sl = slice(s * B, (s + 1) * B)
                    nc.tensor.matmul(
                        pl[:][:, sl], wk[:], hk[:][:, sl],
                        start=k == 0, stop=k == 1,
                    )
            # ex = exp(logits) in bf16; doubles as the logits output
            # (host takes ln).
            ex = exq.tile([128, BB], bf16)
            nc.scalar.activation(ex[:], pl[:], AF.Exp, bias=b2t[:])
            nc.sync.dma_start(eo_d.ap()[:, p["mb"] : p["mb"] + BB], ex[:])
            cs = csp.tile([128, BB + 1], f32)
            init1 = 0.0 if b == 0 else st[b - 1]["cs"][:][:, BB - 1 : BB]
            nc.vector.tensor_tensor_scan(
                out=cs[:][:, 0:BB],
                data0=p["cmb"][:][:, 0:BB],
                data1=ex[:],
                initial=init1,
                op0=OP.mult,
                op1=OP.add,
            )
            p["ex"] = ex
            p["cs"] = cs

        for i in range(NB + 2):
            if i < NB:
                stage1(i)
            if 1 <= i <= NB:
                stage2(i - 1)
            if i >= 2:
                emit_tail(st[i - 2], st.get(i - 1))
                del st[i - 2]["h0"], st[i - 2]["h1"]

    nc.compile()
    return nc


def _get_nc(use_f32r=None):
    key = "nc"
    if key not in _NC_CACHE:
        _NC_CACHE[key] = _build_nc()
    return _NC_CACHE[key]


def make_in_maps(H, batch, W1, b1, W2, b2):
    """Shard the full inputs into 8 per-core input maps."""
    import ml_dtypes

    bf16 = ml_dtypes.bfloat16
    H = np.ascontiguousarray(np.asarray(H, dtype=np.float32))
    batch = np.asarray(batch)
    W1 = np.asarray(W1, dtype=bf16)
    b1 = np.asarray(b1, dtype=np.float32).reshape(FEAT, 1)
    W2 = np.asarray(W2, dtype=bf16)
    b2 = np.asarray(b2, dtype=np.float32).reshape(CLS, 1)

    cuts = np.searchsorted(batch, np.arange(0, NUM_SEGMENTS + 1, SEG_PER_CORE))
    in_maps = []
    counts = []
    for c in range(NCORES):
        s, e = int(cuts[c]), int(cuts[c + 1])
        cnt = e - s
        assert cnt <= MPAD, f"shard {c} has {cnt} rows > MPAD={MPAD}"
        counts.append(cnt)
        ht = np.zeros((2, 128, MPAD), bf16)
        ht[0, :, :cnt] = H[s:e, 0:128].T
        ht[1, :, :cnt] = H[s:e, 128:256].T
        seg = batch[s:e]
        same = np.zeros(cnt, np.float32)
        if cnt > 1:
            same[1:] = (seg[1:] == seg[:-1]).astype(np.float32)
        # the windowed backward scan requires every real segment to be
        # shorter than WIN
        starts = np.flatnonzero(same == 0)
        if starts.size:
            seg_lens = np.diff(np.r_[starts, cnt])
            assert seg_lens.max() <= WIN, (
                f"segment length {seg_lens.max()} exceeds scan window {WIN}"
            )
        cm = np.zeros(MPAD + W + 1, np.float32)
        cm[:cnt] = same
        if cnt < MPAD:
            cm[cnt] = 0.0
            cm[cnt + 1 : MPAD] = 1.0
        cm[MPAD] = 0.0
        cm[MPAD + 1 :] = 1.0
        in_maps.append(
            {
                "ht": ht,
                "w1": W1,
                "w2": W2,
                "b1": b1,
                "b2": b2,
                # pre-broadcast the mask row to all 128 partitions
                "cm": np.ascontiguousarray(
                    np.broadcast_to(
                        cm.astype(bf16).reshape(1, MPAD + W + 1),
                        (128, MPAD + W + 1),
                    )
                ),
            }
        )
    return in_maps, counts


def assemble_outputs(results, counts, out_dtype=np.float32):
    logits = np.empty((sum(counts), CLS), out_dtype)
    probs = np.empty((sum(counts), CLS), out_dtype)
    off = 0
    for c in range(NCORES):
        cnt = counts[c]
        ex = results[c]["eo"][:, :cnt].T.astype(np.float32)
        np.log(ex, out=logits[off : off + cnt])
        probs[off : off + cnt] = results[c]["pt"][:, :cnt].T.astype(out_dtype)
        off += cnt
    return logits, probs


def _axon_devices():
    import jax

    last_err = None
    for plat in ("axon", "neuron"):
        try:
            devs = jax.devices(plat)
            if devs:
                return devs
        except RuntimeError as e:
            last_err = e
    devs = jax.devices()
    if len(devs) >= NCORES and devs[0].platform not in ("cpu",):
        return devs
    raise RuntimeError(f"no axon/neuron devices visible: {last_err}")


def _get_exec(nc):
    """Build (once) a sharded jitted executable over the 8 neuron cores plus
    the metadata needed to call it. Mirrors bass2jax.run_bass_via_pjrt but
    with an explicit device list and a reusable callable."""
    key = ("exec", id(nc))
    if key in _NC_CACHE:
        return _NC_CACHE[key]
    import jax
    from jax.sharding import Mesh, NamedSharding, PartitionSpec
    from jax.experimental.shard_map import shard_map

    from concourse import bass2jax
    import concourse.mybir as mybir

    bass2jax.install_neuronx_cc_hook()
    partition_name = nc.partition_id_tensor.name if nc.partition_id_tensor else None
    in_names, out_names, out_avals = [], [], []
    for alloc in nc.m.functions[0].allocations:
        if not isinstance(alloc, mybir.MemoryLocationSet):
            continue
        name = alloc.memorylocations[0].name
        if alloc.kind == "ExternalInput":
            if name != partition_name:
                in_names.append(name)
        elif alloc.kind == "ExternalOutput":
            out_names.append(name)
            out_avals.append(
                jax.core.ShapedArray(tuple(alloc.tensor_shape), mybir.dt.np(alloc.dtype))
            )
    n_params = len(in_names)
    all_in_names = tuple(in_names) + tuple(out_names)
    if partition_name is not None:
        all_in_names = all_in_names + (partition_name,)

    def _body(*args):
        operands = list(args)
        if partition_name is not None:
            operands.append(bass2jax.partition_id_tensor())
        return tuple(
            bass2jax._bass_exec_p.bind(
                *operands,
                out_avals=tuple(out_avals),
                in_names=all_in_names,
                out_names=tuple(out_names),
                lowering_input_output_aliases=(),
                sim_require_finite=True,
                sim_require_nnan=True,
                nc=nc,
            )
        )

    devices = _axon_devices()[:NCORES]
    mesh = Mesh(np.asarray(devices), ("core",))
    nout = len(out_names)
    sharded = jax.jit(
        shard_map(
            _body,
            mesh=mesh,
            in_specs=(PartitionSpec("core"),) * (n_params + nout),
            out_specs=(PartitionSpec("core"),) * nout,
            check_rep=False,
        ),
        donate_argnums=tuple(range(n_params, n_params + nout)),
        keep_unused=True,
    )
    info = dict(
        fn=sharded,
        in_names=in_names,
        out_names=out_names,
        out_avals=out_avals,
        sharding=NamedSharding(mesh, PartitionSpec("core")),
    )
    _NC_CACHE[key] = info
    return info


def stack_inputs(ex, in_maps):
    """Concatenate the per-core input maps along dim 0 in exec input order."""
    return [
        np.concatenate([np.asarray(in_maps[c][n]) for c in range(NCORES)], axis=0)
        for n in ex["in_names"]
    ]


def run_spmd(nc, in_maps):
    """Run the bass module on the 8 cores; returns per-core result dicts."""
    import jax

    ex = _get_exec(nc)
    concat_in = stack_inputs(ex, in_maps)
    # device_put with the mesh sharding so the per-call execution does not
    # re-slice/scatter the inputs across the 8 cores.
    dev_in = [jax.device_put(a, ex["sharding"]) for a in concat_in]
    zeros = [
        jax.device_put(
            np.zeros((NCORES * av.shape[0], *av.shape[1:]), av.dtype), ex["sharding"]
        )
        for av in ex["out_avals"]
    ]
    outs = ex["fn"](*dev_in, *zeros)
    return [
        {
            name: np.asarray(outs[i]).reshape(NCORES, *ex["out_avals"][i].shape)[c]
            for i, name in enumerate(ex["out_names"])
        }
        for c in range(NCORES)
    ]


def kernel(H, batch, num_segments, W1, b1, W2, b2):
    assert int(num_segments) == NUM_SEGMENTS
    nc = _get_nc()
    in_maps, counts = make_in_maps(H, batch, W1, b1, W2, b2)
    results = run_spmd(nc, in_maps)
    logits, probs = assemble_outputs(results, counts)
    return logits, probs


if __name__ == "__main__":
    rng = np.random.default_rng(0)
    H = rng.standard_normal((N_NODES, FEAT), dtype=np.float32)
    batch = np.sort(rng.integers(0, NUM_SEGMENTS, N_NODES))
    W1 = rng.uniform(-0.0625, 0.0625, (FEAT, FEAT)).astype(np.float32)
    b1 = rng.uniform(-0.0625, 0.0625, FEAT).astype(np.float32)
    W2 = rng.uniform(-0.0625, 0.0625, (FEAT, CLS)).astype(np.float32)
    b2 = rng.uniform(-0.0625, 0.0625, FEAT // 2).astype(np.float32)
    logits, probs = kernel(H, batch, NUM_SEGMENTS, W1, b1, W2, b2)
    print("ok", logits.shape, probs.shape)


# revision 14
# speedup vs baseline: 1.1704x; 1.1704x over previous
"""Trainium2 Bass kernel for nn_PredictionHead (MLP + segment softmax).

Strategy (8 NeuronCores, data-parallel over nodes):
  - Shard the 500k nodes at segment-id boundaries (segments [256c, 256(c+1))
    go to core c) so every segment's rows live on exactly one core.
  - Each core computes in the TRANSPOSED domain (features/classes on the
    partition axis, nodes on the free axis), so the MLP matmuls need no
    on-chip transposes: the host supplies H^T once, cast to bf16.
      h^T   = relu(W1^T @ H^T + b1)        (bf16 matmuls, f32 PSUM)
      ex^T  = exp(W2^T @ h^T + b2)         (ACT Exp, bf16 out)
  - The kernel outputs ex^T (bf16); the host recovers logits = ln(ex)
    exactly enough (|d logit| ~ 2^-9) and skips an on-chip bias-add pass.
  - Segment sums of ex along the node axis are computed with masked
    tensor_tensor_scan ops on the vector engine (segmented prefix sum,
    then a backward masked max-scan that spreads each segment's total over
    the segment), chained across 1024-column blocks; a one-block-lag carry
    fixes segments that straddle a block boundary.
  - rc = 1/spread on the scalar engine (LUT reciprocal, ~1e-6 rel);
    probs^T = ex^T * rc on gpsimd (bf16).
  - Host un-transposes the two outputs and concatenates the shards.

Engine budget per 1024-column block (measured costs):
  DVE    scan1 2.35us + scan2 3.02us            = 5.4us   <- critical path
  ACT    relu 2x1.12 + exp 1.12 + recip 1.12    = 4.5us
  GpSimd mask bcast 2.3 + mul 2.3 + copy 0.45   = 5.1us
  PE     12 matmul x 512 rows + 6 ldweights     ~ 3.5-4us
  DMA    ~1.1 MB                                ~ 2.9us
"""

import os
import sys

import numpy as np

if "/opt/trn_rl_repo" not in sys.path:
    sys.path.insert(0, "/opt/trn_rl_repo")

# Make sure the axon (neuron) PJRT platform stays reachable even if the
# embedding process pinned JAX_PLATFORMS=cpu for the jax reference.
_jp = os.environ.get("JAX_PLATFORMS")
if _jp and "axon" not in _jp and "jax" not in sys.modules:
    os.environ["JAX_PLATFORMS"] = _jp + ",axon"

N_NODES = 500_000
FEAT = 256
CLS = 128
NUM_SEGMENTS = 2048
NCORES = 8
SEG_PER_CORE = NUM_SEGMENTS // NCORES
B = 512       # matmul / PSUM bank sub-block width
BB = 1024     # columns processed per iteration
MPAD = 63488  # 62 iterations of 1024; max shard for the reference seed is 62846
NB = MPAD // BB
WIN = 352     # backward-scan lookahead; must exceed the max segment length
W = BB + WIN

_NC_CACHE = {}


def _recip_fast(nc, out, in_):
    """reciprocal_approx_fast with a bf16 output (bypasses the f32-out
    assert; the DVE write stage downcasts). ~51 ULP in f32, so the bf16
    rounding dominates -- fine at this problem's 2e-2 tolerance."""
    from concourse.dve_ops import (
        RECIP_APPROX_FAST_CONSTS,
        RECIPROCAL_APPROX_FAST,
    )

    c = RECIP_APPROX_FAST_CONSTS
    return nc.vector._custom_dve(
        RECIPROCAL_APPROX_FAST,
        out=out,
        in0=in_,
        s0=c["s0"],
        s1=c["s1"],
        imm2=c["imm2"],
    )


def _build_nc(mul_on_gpsimd=False):
    from contextlib import ExitStack

    import concourse.bacc as bacc
    import concourse.mybir as mybir
    import concourse.tile as tile

    f32 = mybir.dt.float32
    bf16 = mybir.dt.bfloat16
    AF = mybir.ActivationFunctionType
    OP = mybir.AluOpType

    nc = bacc.Bacc("TRN2", target_bir_lowering=False, debug=False)
    # ht is laid out [2, 128, MPAD]: the two 128-row k-chunks of H^T stacked,
    # so one DMA per iteration fetches both chunks of a column block.
    ht_d = nc.dram_tensor("ht", [2, 128, MPAD], bf16, kind="ExternalInput")
    w1_d = nc.dram_tensor("w1", [FEAT, FEAT], bf16, kind="ExternalInput")
    w2_d = nc.dram_tensor("w2", [FEAT, CLS], bf16, kind="ExternalInput")
    b1_d = nc.dram_tensor("b1", [FEAT, 1], f32, kind="ExternalInput")
    b2_d = nc.dram_tensor("b2", [CLS, 1], f32, kind="ExternalInput")
    # mask ships pre-broadcast to 128 partitions (bf16, tiny vs H) so no
    # on-chip partition broadcast is needed.
    cm_d = nc.dram_tensor("cm", [128, MPAD + W + 1], bf16, kind="ExternalInput")
    eo_d = nc.dram_tensor("eo", [CLS, MPAD], bf16, kind="ExternalOutput")
    pt_d = nc.dram_tensor("pt", [CLS, MPAD], bf16, kind="ExternalOutput")

    with ExitStack() as ctx:
        tc = ctx.enter_context(tile.TileContext(nc))
        consts = ctx.enter_context(tc.tile_pool(name="consts", bufs=1))
        htp = ctx.enter_context(tc.tile_pool(name="htp", bufs=4))
        hp = ctx.enter_context(tc.tile_pool(name="hp", bufs=6))
        psh = ctx.enter_context(tc.tile_pool(name="psh", bufs=1, space="PSUM"))
        psl = ctx.enter_context(tc.tile_pool(name="psl", bufs=2, space="PSUM"))
        exq = ctx.enter_context(tc.tile_pool(name="exq", bufs=4))
        csp = ctx.enter_context(tc.tile_pool(name="csp", bufs=3))
        mbc = ctx.enter_context(tc.tile_pool(name="mbc", bufs=4))
        prescan = ctx.enter_context(tc.tile_pool(name="prescan", bufs=2))
        xpp = ctx.enter_context(tc.tile_pool(name="xpp", bufs=3))
        rcp = ctx.enter_context(tc.tile_pool(name="rcp", bufs=3))
        prp = ctx.enter_context(tc.tile_pool(name="prp", bufs=3))

        w1k0 = consts.tile([128, FEAT], bf16)
        nc.sync.dma_start(w1k0[:], w1_d.ap()[0:128, :])
        w1k1 = consts.tile([128, FEAT], bf16)
        nc.sync.dma_start(w1k1[:], w1_d.ap()[128:256, :])
        w2k0 = consts.tile([128, CLS], bf16)
        nc.sync.dma_start(w2k0[:], w2_d.ap()[0:128, :])
        w2k1 = consts.tile([128, CLS], bf16)
        nc.sync.dma_start(w2k1[:], w2_d.ap()[128:256, :])
        b1a = consts.tile([128, 1], f32)
        nc.sync.dma_start(b1a[:], b1_d.ap()[0:128, :])
        b1b = consts.tile([128, 1], f32)
        nc.sync.dma_start(b1b[:], b1_d.ap()[128:256, :])
        b2t = consts.tile([128, 1], f32)
        nc.sync.dma_start(b2t[:], b2_d.ap()[:, :])

        def emit_tail(p, nxt):
            # Backward masked max-scan spreads each segment's total (csum at
            # its last column) over the segment. The carry INTO this block's
            # last column comes from a short pre-scan over the first WIN
            # columns of the NEXT block; it is injected as an extra data
            # element (cs column BB) rather than via `initial` -- a reversed
            # scan with an AP initial runs ~1.7x slower on hardware.
            if nxt is not None:
                pre = prescan.tile([128, WIN], f32)
                nc.vector.tensor_tensor_scan(
                    out=pre[:][:, ::-1],
                    data0=nxt["cmb"][:][:, 1 : WIN + 1][:, ::-1],
                    data1=nxt["cs"][:][:, 0:WIN][:, ::-1],
                    initial=0.0,
                    op0=OP.mult,
                    op1=OP.max,
                )
                nc.vector.tensor_copy(p["cs"][:][:, BB : BB + 1], pre[:][:, 0:1])
            else:
                nc.vector.memset(p["cs"][:][:, BB : BB + 1], 0.0)
            xpd = xpp.tile([128, BB + 1], f32)
            nc.vector.tensor_tensor_scan(
                out=xpd[:][:, ::-1],
                data0=p["cmb"][:][:, 1 : BB + 2][:, ::-1],
                data1=p["cs"][:][:, ::-1],
                initial=0.0,
                op0=OP.mult,
                op1=OP.max,
            )
            rc = rcp.tile([128, BB], bf16)
            _recip_fast(nc, rc[:], xpd[:][:, 0:BB])
            pr = prp.tile([128, BB], bf16)
            if mul_on_gpsimd:
                nc.gpsimd.tensor_mul(pr[:], p["ex"][:], rc[:])
            else:
                nc.vector.tensor_mul(pr[:], p["ex"][:], rc[:])
            nc.sync.dma_start(pt_d.ap()[:, p["mb"] : p["mb"] + BB], pr[:])

        # Software-pipelined schedule: in iteration i the tensor engine runs
        # MM1(i) then MM2(i-1) -- MM2's relu dependency is already satisfied,
        # so the PE never head-of-line blocks (keeps the HAM clock warm).
        st = {}

        def stage1(b):
            mb = b * BB
            htb = htp.tile([128, 2, BB], bf16)
            nc.sync.dma_start(
                htb[:], ht_d.ap()[:, :, mb : mb + BB].rearrange("k p m -> p k m")
            )
            cmb = mbc.tile([128, BB + 2], bf16)
            nc.sync.dma_start(cmb[:], cm_d.ap()[:, mb : mb + BB + 2])
            ht0 = htb[:][:, 0, :]
            ht1 = htb[:][:, 1, :]
            ph0 = psh.tile([128, BB], f32, tag="ph0")
            ph1 = psh.tile([128, BB], f32, tag="ph1")
            for c, ph in ((0, ph0), (1, ph1)):
                cs_ = slice(128 * c, 128 * (c + 1))
                for k, (wk, htk) in enumerate(((w1k0, ht0), (w1k1, ht1))):
                    for s in range(2):
                        sl = slice(s * B, (s + 1) * B)
                        nc.tensor.matmul(
                            ph[:][:, sl], wk[:][:, cs_], htk[:, sl],
                            start=k == 0, stop=k == 1,
                        )
            h0 = hp.tile([128, BB], bf16)
            nc.scalar.activation(h0[:], ph0[:], AF.Relu, bias=b1a[:])
            h1 = hp.tile([128, BB], bf16)
            nc.scalar.activation(h1[:], ph1[:], AF.Relu, bias=b1b[:])
            st[b] = dict(h0=h0, h1=h1, cmb=cmb, mb=mb)

        def stage2(b):
            p = st[b]
            pl = psl.tile([128, BB], f32)
            for k, (wk, hk) in enumerate(((w2k0, p["h0"]), (w2k1, p["h1"]))):
                for s in range(2):
                    sl = slice(s * B, (s + 1) * B)
                    nc.tensor.matmul(
                        pl[:][:, sl], wk[:], hk[:][:, sl],
                        start=k == 0, stop=k == 1,
                    )
            # ex = exp(logits) in bf16; doubles as the logits output
            # (host takes ln).
            ex = exq.tile([128, BB], bf16)
            nc.scalar.activation(ex[:], pl[:], AF.Exp, bias=b2t[:])
            nc.sync.dma_start(eo_d.ap()[:, p["mb"] : p["mb"] + BB], ex[:])
            cs = csp.tile([128, BB + 1], f32)
            init1 = 0.0 if b == 0 else st[b - 1]["cs"][:][:, BB - 1 : BB]
            nc.vector.tensor_tensor_scan(
                out=cs[:][:, 0:BB],
                data0=p["cmb"][:][:, 0:BB],
                data1=ex[:],
                initial=init1,
                op0=OP.mult,
                op1=OP.add,
            )
            p["ex"] = ex
            p["cs"] = cs

        for i in range(NB + 2):
            if i < NB:
                stage1(i)
            if 1 <= i <= NB:
                stage2(i - 1)
            if i >= 2:
                emit_tail(st[i - 2], st.get(i - 1))
                del st[i - 2]["h0"], st[i - 2]["h1"]

    nc.compile()
    return nc


def _get_nc(use_f32r=None):
    key = "nc"
    if key not in _NC_CACHE:
        _NC_CACHE[key] = _build_nc()
    return _NC_CACHE[key]


def make_in_maps(H, batch, W1, b1, W2, b2):
    """Shard the full inputs into 8 per-core input maps."""
    import ml_dtypes

    bf16 = ml_dtypes.bfloat16
    H = np.ascontiguousarray(np.asarray(H, dtype=np.float32))
    batch = np.asarray(batch)
    W1 = np.asarray(W1, dtype=bf16)
    b1 = np.asarray(b1, dtype=np.float32).reshape(FEAT, 1)
    W2 = np.asarray(W2, dtype=bf16)
    b2 = np.asarray(b2, dtype=np.float32).reshape(CLS, 1)

    cuts = np.searchsorted(batch, np.arange(0, NUM_SEGMENTS + 1, SEG_PER_CORE))
    in_maps = []
    counts = []
    for c in range(NCORES):
        s, e = int(cuts[c]), int(cuts[c + 1])
        cnt = e - s
        assert cnt <= MPAD, f"shard {c} has {cnt} rows > MPAD={MPAD}"
        counts.append(cnt)
        ht = np.zeros((2, 128, MPAD), bf16)
        ht[0, :, :cnt] = H[s:e, 0:128].T
        ht[1, :, :cnt] = H[s:e, 128:256].T
        seg = batch[s:e]
        same = np.zeros(cnt, np.float32)
        if cnt > 1:
            same[1:] = (seg[1:] == seg[:-1]).astype(np.float32)
        # the windowed backward scan requires every real segment to be
        # shorter than WIN
        starts = np.flatnonzero(same == 0)
        if starts.size:
            seg_lens = np.diff(np.r_[starts, cnt])
            assert seg_lens.max() <= WIN, (
                f"segment length {seg_lens.max()} exceeds scan window {WIN}"
            )
        cm = np.zeros(MPAD + W + 1, np.float32)
        cm[:cnt] = same
        if cnt < MPAD:
            cm[cnt] = 0.0
            cm[cnt + 1 : MPAD] = 1.0
        cm[MPAD] = 0.0
        cm[MPAD + 1 :] = 1.0
        in_maps.append(
            {
                "ht": ht,
                "w1": W1,
                "w2": W2,
                "b1": b1,
                "b2": b2,
                # pre-broadcast the mask row to all 128 partitions
                "cm": np.ascontiguousarray(
                    np.broadcast_to(
                        cm.astype(bf16).reshape(1, MPAD + W + 1),
                        (128, MPAD + W + 1),
                    )
                ),
            }
        )
    return in_maps, counts


def assemble_outputs(results, counts, out_dtype=np.float32):
    logits = np.empty((sum(counts), CLS), out_dtype)
    probs = np.empty((sum(counts), CLS), out_dtype)
    off = 0
    for c in range(NCORES):
        cnt = counts[c]
        ex = results[c]["eo"][:, :cnt].T.astype(np.float32)
        np.log(ex, out=logits[off : off + cnt])
        probs[off : off + cnt] = results[c]["pt"][:, :cnt].T.astype(out_dtype)
        off += cnt
    return logits, probs


def _axon_devices():
    import jax

    last_err = None
    for plat in ("axon", "neuron"):
        try:
            devs = jax.devices(plat)
            if devs:
                return devs
        except RuntimeError as e:
            last_err = e
    devs = jax.devices()
    if len(devs) >= NCORES and devs[0].platform not in ("cpu",):
        return devs
    raise RuntimeError(f"no axon/neuron devices visible: {last_err}")


def _get_exec(nc):
    """Build (once) a sharded jitted executable over the 8 neuron cores plus
    the metadata needed to call it. Mirrors bass2jax.run_bass_via_pjrt but
    with an explicit device list and a reusable callable."""
    key = ("exec", id(nc))
    if key in _NC_CACHE:
        return _NC_CACHE[key]
    import jax
    from jax.sharding import Mesh, NamedSharding, PartitionSpec
    from jax.experimental.shard_map import shard_map

    from concourse import bass2jax
    import concourse.mybir as mybir

    bass2jax.install_neuronx_cc_hook()
    partition_name = nc.partition_id_tensor.name if nc.partition_id_tensor else None
    in_names, out_names, out_avals = [], [], []
    for alloc in nc.m.functions[0].allocations:
        if not isinstance(alloc, mybir.MemoryLocationSet):
            continue
        name = alloc.memorylocations[0].name
        if alloc.kind == "ExternalInput":
            if name != partition_name:
                in_names.append(name)
        elif alloc.kind == "ExternalOutput":
            out_names.append(name)
            out_avals.append(
                jax.core.ShapedArray(tuple(alloc.tensor_shape), mybir.dt.np(alloc.dtype))
            )
    n_params = len(in_names)
    all_in_names = tuple(in_names) + tuple(out_names)
    if partition_name is not None:
        all_in_names = all_in_names + (partition_name,)

    def _body(*args):
        operands = list(args)
        if partition_name is not None:
            operands.append(bass2jax.partition_id_tensor())
        return tuple(
            bass2jax._bass_exec_p.bind(
                *operands,
                out_avals=tuple(out_avals),
                in_names=all_in_names,
                out_names=tuple(out_names),
                lowering_input_output_aliases=(),
                sim_require_finite=True,
                sim_require_nnan=True,
                nc=nc,
            )
        )

    devices = _axon_devices()[:NCORES]
    mesh = Mesh(np.asarray(devices), ("core",))
    nout = len(out_names)
    sharded = jax.jit(
        shard_map(
            _body,
            mesh=mesh,
            in_specs=(PartitionSpec("core"),) * (n_params + nout),
            out_specs=(PartitionSpec("core"),) * nout,
            check_rep=False,
        ),
        donate_argnums=tuple(range(n_params, n_params + nout)),
        keep_unused=True,
    )
    info = dict(
        fn=sharded,
        in_names=in_names,
        out_names=out_names,
        out_avals=out_avals,
        sharding=NamedSharding(mesh, PartitionSpec("core")),
    )
    _NC_CACHE[key] = info
    return info


def stack_inputs(ex, in_maps):
    """Concatenate the per-core input maps along dim 0 in exec input order."""
    return [
        np.concatenate([np.asarray(in_maps[c][n]) for c in range(NCORES)], axis=0)
        for n in ex["in_names"]
    ]


def run_spmd(nc, in_maps):
    """Run the bass module on the 8 cores; returns per-core result dicts."""
    import jax

    ex = _get_exec(nc)
    concat_in = stack_inputs(ex, in_maps)
    # device_put with the mesh sharding so the per-call execution does not
    # re-slice/scatter the inputs across the 8 cores.
    dev_in = [jax.device_put(a, ex["sharding"]) for a in concat_in]
    zeros = [
        jax.device_put(
            np.zeros((NCORES * av.shape[0], *av.shape[1:]), av.dtype), ex["sharding"]
        )
        for av in ex["out_avals"]
    ]
    outs = ex["fn"](*dev_in, *zeros)
    return [
        {
            name: np.asarray(outs[i]).reshape(NCORES, *ex["out_avals"][i].shape)[c]
            for i, name in enumerate(ex["out_names"])
        }
        for c in range(NCORES)
    ]


def kernel(H, batch, num_segments, W1, b1, W2, b2):
    assert int(num_segments) == NUM_SEGMENTS
    nc = _get_nc()
    in_maps, counts = make_in_maps(H, batch, W1, b1, W2, b2)
    results = run_spmd(nc, in_maps)
    logits, probs = assemble_outputs(results, counts)
    return logits, probs


if __name__ == "__main__":
    rng = np.random.default_rng(0)
    H = rng.standard_normal((N_NODES, FEAT), dtype=np.float32)
    batch = np.sort(rng.integers(0, NUM_SEGMENTS, N_NODES))
    W1 = rng.uniform(-0.0625, 0.0625, (FEAT, FEAT)).astype(np.float32)
    b1 = rng.uniform(-0.0625, 0.0625, FEAT).astype(np.float32)
    W2 = rng.uniform(-0.0625, 0.0625, (FEAT, CLS)).astype(np.float32)
    b2 = rng.uniform(-0.0625, 0.0625, FEAT // 2).astype(np.float32)
    logits, probs = kernel(H, batch, NUM_SEGMENTS, W1, b1, W2, b2)
    print("ok", logits.shape, probs.shape)
